# revision 41
# baseline (speedup 1.0000x reference)
"""Fused 2-layer KAN for Trainium2, data-parallel across 8 NeuronCores.

Math: with G=3 grid points the spline basis is piecewise-linear in x, so each
KAN layer collapses to a small dense matmul over 3 cheap feature maps:

    out = bias + silu(x) @ Wb + u @ P1 + C @ (P2 - P1)
      u = clip(x, -1, 1),  C = max(u, 0)
      Wb = imp*bw;  T = imp*sw*cp;  P1 = T@(bv1-bv0);  P2 = T@(bv2-bv1)
      bias_j = sum_i T[i,j,:] @ bv1

All K=5 spline control points fold into P1/P2/bias on the host (O(I*J*K) work).

Device schedule: software-pipelined at skew 2 so no engine ever waits on a
same-iteration cross-engine product.  Iteration m emits:
  x-DMA(m+2) [GPSIMD SWDGE cast] -> PE transpose(m+1) -> L1 feature maps(m+1)
  [DVE u/c, ACT silu] -> L1 matmul(m) -> L2 feature maps(m-1) [single
  1024-wide ACT/DVE ops from a 2-bank PSUM h tile] -> bias ones-matmul +
  L2 matmul(m-1) -> PSUM->SBUF copy on GPSIMD -> out DMA (sync HWDGE).
"""

import os
import sys
from contextlib import ExitStack

import numpy as np
import ml_dtypes

for _p in ("/opt/trn_rl_repo",):
    if _p not in sys.path and os.path.isdir(_p):
        sys.path.insert(0, _p)

import concourse.bass as bass
import concourse.tile as tile
from concourse import bacc, mybir
from concourse.bass_utils import run_bass_kernel_spmd
from concourse.masks import make_identity

F32 = mybir.dt.float32
BF16 = mybir.dt.bfloat16
BF = ml_dtypes.bfloat16

N_CORES = 8
D0, D1, D2 = 64, 128, 64
K, DEG, G, LO, HI = 5, 3, 3, -1.0, 1.0
MACRO = 1024  # batch rows per device macro-iteration

_nc_cache = {}


def _basis_table():
    knots = np.linspace(LO - DEG * 0.1, HI + DEG * 0.1, K + DEG + 1)
    grid = np.linspace(LO, HI, G)
    bv = np.zeros((G, K), dtype=np.float32)
    for i in range(K):
        center = (knots[i + DEG // 2] + knots[i + DEG // 2 + 1]) / 2.0
        width = (knots[i + DEG + 1] - knots[i]) / 2.0
        bv[:, i] = np.exp(-(((grid - center) / width) ** 2))
    bv = bv / (bv.sum(axis=1, keepdims=True) + 1e-6)
    return bv


def _prep_consts(cp0, bw0, sw0, imp0, cp1, bw1, sw1, imp1):
    f8 = np.float64
    bv = _basis_table().astype(f8)
    d1, d2 = bv[1] - bv[0], bv[2] - bv[1]

    def fold(cp, bw, sw, imp):
        T = imp.astype(f8)[:, :, None] * sw.astype(f8)[:, :, None] * cp.astype(f8)
        Wb = imp.astype(f8) * bw.astype(f8)
        return Wb, T @ d1, T @ d2, (T @ bv[1]).sum(axis=0)

    Wb0, P10, P20, b1 = fold(cp0, bw0, sw0, imp0)
    Wb1, P11, P21, b2 = fold(cp1, bw1, sw1, imp1)
    bias2_eff = b2 + b1 @ P21

    w1 = np.stack([Wb0, P10, P20 - P10], axis=0)  # [3, 64, 128] lhsT chunks
    w1 = np.concatenate([w1, w1], axis=1)  # duplicate rows for partitions 64-127
    w1 = np.ascontiguousarray(w1.transpose(1, 0, 2)).reshape(128, 384)
    w2 = np.stack([Wb1, P11, P21 - P11], axis=0)  # [3, 128, 64] rhs chunks
    w2 = np.ascontiguousarray(w2.transpose(1, 0, 2)).reshape(128, 192)

    return {
        "wpk": np.concatenate([w1, w2], axis=1).astype(BF),  # [128, 576]
        "spk": np.stack(
            [b1, -1.0 - b1, 1.0 - b1, -b1], axis=1
        ).astype(np.float32),  # [128, 4] = b1|s1|s2|nb1
        "b2row": np.tile(bias2_eff, 8).astype(BF).reshape(1, 512),
    }


def _build(rows):
    assert rows % MACRO == 0
    nc = bacc.Bacc(
        "TRN2",
        target_bir_lowering=False,
        debug=False,
        enable_asserts=False,
        num_devices=N_CORES,
    )
    xd = nc.dram_tensor("x", [rows, D0], F32, kind="ExternalInput")
    wpkd = nc.dram_tensor("wpk", [128, 576], BF16, kind="ExternalInput")
    spkd = nc.dram_tensor("spk", [128, 4], F32, kind="ExternalInput")
    b2d = nc.dram_tensor("b2row", [1, 512], BF16, kind="ExternalInput")
    outd = nc.dram_tensor("out", [rows, D2], F32, kind="ExternalOutput")

    n_macro = rows // MACRO
    MAX, MIN = mybir.AluOpType.max, mybir.AluOpType.min
    SILU = mybir.ActivationFunctionType.Silu

    with tile.TileContext(nc) as tc, ExitStack() as ctx:
        consts = ctx.enter_context(tc.tile_pool(name="consts", bufs=1))
        xin = ctx.enter_context(tc.tile_pool(name="xin", bufs=8))
        f1 = ctx.enter_context(tc.tile_pool(name="f1", bufs=3))
        f2 = ctx.enter_context(tc.tile_pool(name="f2", bufs=2))
        osb = ctx.enter_context(tc.tile_pool(name="osb", bufs=3))
        ps_x = ctx.enter_context(tc.tile_pool(name="ps_x", bufs=1, space="PSUM"))
        ps_h = ctx.enter_context(tc.tile_pool(name="ps_h", bufs=2, space="PSUM"))
        ps_o = ctx.enter_context(tc.tile_pool(name="ps_o", bufs=3, space="PSUM"))

        wpk = consts.tile([128, 576], BF16)
        nc.sync.dma_start(wpk, wpkd.ap())
        spk = consts.tile([128, 4], F32)
        nc.sync.dma_start(spk, spkd.ap())
        b2r = consts.tile([1, 512], BF16)
        nc.sync.dma_start(b2r, b2d.ap())

        # PE pre-warm: dummy matmuls on a memset tile (no iota dependency)
        # while DMAs land, so the HAM clock gate opens (1.2 -> 2.4 GHz)
        # before the first real matmul issues.
        wz = consts.tile([128, 128], BF16)
        nc.vector.memset(wz, 0.0)
        warm = ps_o.tile([128, 8, 64], F32, tag="po")

        def warm_pe(n):
            for _ in range(n):
                nc.tensor.matmul(warm[:, 0:2], wz, wz, start=True, stop=True)

        warm_pe(18)

        ident = consts.tile([128, 128], BF16)
        make_identity(nc, ident)
        ones = consts.tile([1, 128], BF16)
        nc.vector.memset(ones, 1.0)
        b1, s1, s2, nb1 = (spk[:, i : i + 1] for i in range(4))
        w1c = [wpk[:, c * 128 : (c + 1) * 128] for c in range(3)]
        w2c = [wpk[:, 384 + c * 64 : 384 + (c + 1) * 64] for c in range(3)]

        xts, pxs, u1s, sl1s, c1s, hs = {}, {}, {}, {}, {}, {}

        def xt_dma_single(m):
            # Macros 0/1 load alone (2 KB descriptors) so the very first
            # transpose waits on a 256 KB transfer, not 512 KB.
            # xt[p, j, f] = x[base + 8p + j, f]
            xt = xin.tile([128, 8, 64], BF16, tag="xts")
            src = bass.AP(
                xd, m * MACRO * 64,
                [[8 * 64, 128], [64, 8], [1, 64]],
            )
            nc.gpsimd.dma_start(xt, src)
            xts[("s", m)] = xt

        def xt_dma_pair(k):
            # Load macros 2k and 2k+1 in one SWDGE: partition p reads 16
            # consecutive rows of x = one contiguous 4 KB descriptor (cast to
            # 2 KB bf16).  xt[p, j, f] = x[pair_base + 16p + j, f]; macro
            # 2k uses j=0..7, macro 2k+1 uses j=8..15.  Row "block" g of a
            # macro holds rows {base + 16p + g} (stride-16 permutation),
            # undone by the out DMA with equally-contiguous descriptors.
            xt = xin.tile([128, 16, 64], BF16, tag="xt")
            src = bass.AP(
                xd, k * 2 * MACRO * 64,
                [[16 * 64, 128], [64, 16], [1, 64]],
            )
            nc.gpsimd.dma_start(xt, src)
            xts[k] = xt

        def stage_transpose(m):
            # px[p,q,:] partitions 0-63 = feats of block 2q, partitions
            # 64-127 = feats of block 2q+1; free = 128 rows
            px = ps_x.tile([128, 4, 128], BF16, tag="px")
            if m < 2:
                xt, off = xts[("s", m)], 0
            else:
                xt, off = xts[m // 2], 8 * (m % 2)
            for q in range(4):
                nc.tensor.transpose(px[:, q], xt[:, off + 2 * q : off + 2 * q + 2], ident)
            if m < 2:
                del xts[("s", m)]
            elif m % 2 == 1:
                del xts[m // 2]
            pxs[m] = px

        def stage_feats1(m):
            px = pxs.pop(m)
            u1 = f1.tile([128, 4, 128], BF16, tag="u1")
            nc.vector.tensor_scalar(u1, px, -1.0, 1.0, op0=MAX, op1=MIN)
            sl1 = f1.tile([128, 4, 128], BF16, tag="sl1")
            nc.scalar.activation(sl1, px, SILU)
            c1 = f1.tile([128, 4, 128], BF16, tag="c1")
            nc.vector.tensor_scalar_max(c1, u1, 0.0)
            u1s[m], sl1s[m], c1s[m] = u1, sl1, c1

        def stage_l1(m):
            # L1: two concurrent 64-contraction row-group streams (A=even
            # blocks on partitions 0-63 of h[:, 0:512], B=odd on 512:1024)
            h = ps_h.tile([128, 1024], F32, tag="h")
            u1, sl1, c1 = u1s.pop(m), sl1s.pop(m), c1s.pop(m)
            for i, (c, ft) in enumerate([(1, u1), (0, sl1), (2, c1)]):
                nc.tensor.matmul(
                    h[:, 0:512], w1c[c][0:64], ft[0:64], start=(i == 0), stop=(i == 2)
                )
                nc.tensor.matmul(
                    h[:, 512:1024], w1c[c][64:128], ft[64:128],
                    start=(i == 0), stop=(i == 2),
                )
            hs[m] = h

        maps2 = {}

        def stage_maps2(m):
            h = hs.pop(m)
            # L2 feature maps; sl2 is one 1024-wide op spanning both h banks
            sl2 = f2.tile([128, 1024], BF16, tag="sl2")
            nc.scalar.activation(sl2, h, SILU, bias=b1)
            u2 = f2.tile([128, 1024], BF16, tag="u2")
            nc.vector.tensor_scalar(u2[:, 0:512], h[:, 0:512], s1, s2, op0=MAX, op1=MIN)
            nc.vector.tensor_scalar(u2[:, 512:1024], h[:, 512:1024], s1, s2, op0=MAX, op1=MIN)
            c2 = f2.tile([128, 1024], BF16, tag="c2")
            nc.vector.tensor_scalar_max(c2, u2, nb1)
            maps2[m] = (sl2, u2, c2)

        pos = {}

        def stage_l2mm(m):
            sl2, u2, c2 = maps2.pop(m)
            # bias init via K=1 ones-matmul (sets has_written on the whole
            # bank so the 24 block matmuls accumulate with start=False)
            po = ps_o.tile([128, 8, 64], F32, tag="po")
            nc.tensor.matmul(po, ones, b2r, start=True, stop=False)
            # chunk order by map readiness: u2 first, then sl2, then c2
            for ci, ft in [(1, u2), (0, sl2), (2, c2)]:
                for g in range(8):
                    off = (g % 2) * 512 + (g // 2) * 128
                    nc.tensor.matmul(
                        po[:, g],
                        ft[:, off : off + 128],
                        w2c[ci],
                        start=False,
                        stop=(ci == 2 and g == 7),
                    )
            pos[m] = po

        def stage_out(m):
            po = pos.pop(m)
            ot = osb.tile([128, 8, 64], F32, tag="ot")
            nc.scalar.copy(ot, po)
            # per-partition 8 consecutive out rows = one contiguous 2 KB
            # descriptor; row base is 8p (single-loaded macros) or 16p (pairs)
            if m < 2:
                dst = bass.AP(outd, m * MACRO * 64, [[8 * 64, 128], [64, 8], [1, 64]])
            else:
                dst = bass.AP(
                    outd, ((m // 2) * 2 * MACRO + 8 * (m % 2)) * 64,
                    [[16 * 64, 128], [64, 8], [1, 64]],
                )
            nc.sync.dma_start(dst, ot)

        # Software-pipelined main loop.  Macro 0 runs unskewed (its stage-B
        # right after its L1) so the first output leaves early; the pipeline
        # bubble that builds the steady skew-1 lands in iteration 1, where it
        # is cheap.  Warm matmuls interleave with the first iterations to
        # keep the PE clock hot through fill (PE idle gaps reset the HAM
        # clock ramp back to 1.2 GHz).
        assert n_macro % 2 == 0
        # Macros 0 and 1 load alone; the rest in pairs, prefetched a few
        # iterations ahead so the in-queue drains early.
        xt_dma_single(0)
        xt_dma_single(1)
        for k in range(1, 4):
            xt_dma_pair(k)
        stage_transpose(0)
        warm_pe(8)
        stage_feats1(0)
        warm_pe(6)
        warm_fill = {1: (4, 10), 2: (3, 4), 3: (2, 3)}
        for m in range(n_macro):
            if m in (2, 4, 6, 8):
                xt_dma_pair(m // 2 + 3)
            if m + 1 < n_macro:
                stage_transpose(m + 1)
            if m in warm_fill:
                warm_pe(warm_fill[m][0])
            if m == 0:
                # macro 0 unskewed, and its maps2 emitted before feats1(1)
                # so the DVE/ACT queues reach it without blocking on T(1)
                warm_pe(4)
                stage_l1(0)
                stage_maps2(0)
                warm_pe(6)
                stage_feats1(1)
                stage_l2mm(0)
                continue
            if m + 1 < n_macro:
                stage_feats1(m + 1)
            stage_l1(m)
            if m in warm_fill:
                warm_pe(warm_fill[m][1])
            if m >= 2:
                stage_maps2(m - 1)
                stage_l2mm(m - 1)
                stage_out(m - 2)
        stage_maps2(n_macro - 1)
        stage_l2mm(n_macro - 1)
        stage_out(n_macro - 2)
        # split the final macro's output in half so its first DMA transfer
        # starts as soon as half the copy is done (shorter drain tail)
        mlast = n_macro - 1
        po = pos.pop(mlast)
        base = (mlast // 2) * 2 * MACRO + 8 * (mlast % 2)
        for half in range(2):
            ot = osb.tile([128, 4, 64], F32, tag="oth")
            nc.scalar.copy(ot, po[:, 4 * half : 4 * half + 4])
            dst = bass.AP(
                outd, (base + 4 * half) * 64,
                [[16 * 64, 128], [64, 4], [1, 64]],
            )
            nc.sync.dma_start(dst, ot)

    nc.compile()
    return nc


def _get_nc(rows):
    if rows not in _nc_cache:
        _nc_cache[rows] = _build(rows)
    return _nc_cache[rows]


def kernel(x, cp0, bw0, sw0, imp0, cp1, bw1, sw1, imp1, _trace=False, _trace_kwargs=None):
    x = np.ascontiguousarray(np.asarray(x, dtype=np.float32))
    consts = _prep_consts(
        *[np.asarray(a, dtype=np.float32) for a in (cp0, bw0, sw0, imp0, cp1, bw1, sw1, imp1)]
    )
    rows = x.shape[0] // N_CORES
    nc = _get_nc(rows)
    in_maps = []
    for i in range(N_CORES):
        m = dict(consts)
        m["x"] = x[i * rows : (i + 1) * rows]
        in_maps.append(m)
    res = run_bass_kernel_spmd(
        nc, in_maps, list(range(N_CORES)), trace=_trace, **(_trace_kwargs or {})
    )
    out = np.concatenate([res.results[i]["out"] for i in range(N_CORES)], axis=0)
    if _trace:
        return out, res
    return out


# revision 42
# speedup vs baseline: 1.1613x; 1.1613x over previous
"""Fused 2-layer KAN for Trainium2, data-parallel across 8 NeuronCores.

Math: with G=3 grid points the spline basis is piecewise-linear in x, so each
KAN layer collapses to a small dense matmul over 3 cheap feature maps:

    out = bias + silu(x) @ Wb + u @ P1 + C @ (P2 - P1)
      u = clip(x, -1, 1),  C = max(u, 0)
      Wb = imp*bw;  T = imp*sw*cp;  P1 = T@(bv1-bv0);  P2 = T@(bv2-bv1)
      bias_j = sum_i T[i,j,:] @ bv1

All K=5 spline control points fold into P1/P2/bias on the host (O(I*J*K) work).

Device schedule: software-pipelined at skew 2 so no engine ever waits on a
same-iteration cross-engine product.  Iteration m emits:
  x-DMA(m+2) [GPSIMD SWDGE cast] -> PE transpose(m+1) -> L1 feature maps(m+1)
  [DVE u/c, ACT silu] -> L1 matmul(m) -> L2 feature maps(m-1) [single
  1024-wide ACT/DVE ops from a 2-bank PSUM h tile] -> bias ones-matmul +
  L2 matmul(m-1) -> PSUM->SBUF copy on GPSIMD -> out DMA (sync HWDGE).
"""

import os
import sys
from contextlib import ExitStack

import numpy as np
import ml_dtypes

for _p in ("/opt/trn_rl_repo",):
    if _p not in sys.path and os.path.isdir(_p):
        sys.path.insert(0, _p)

import concourse.bass as bass
import concourse.tile as tile
from concourse import bacc, mybir
from concourse.bass_utils import run_bass_kernel_spmd
from concourse.masks import make_identity

F32 = mybir.dt.float32
BF16 = mybir.dt.bfloat16
BF = ml_dtypes.bfloat16

N_CORES = 8
D0, D1, D2 = 64, 128, 64
K, DEG, G, LO, HI = 5, 3, 3, -1.0, 1.0
MACRO = 1024  # batch rows per device macro-iteration

_nc_cache = {}


def _basis_table():
    knots = np.linspace(LO - DEG * 0.1, HI + DEG * 0.1, K + DEG + 1)
    grid = np.linspace(LO, HI, G)
    bv = np.zeros((G, K), dtype=np.float32)
    for i in range(K):
        center = (knots[i + DEG // 2] + knots[i + DEG // 2 + 1]) / 2.0
        width = (knots[i + DEG + 1] - knots[i]) / 2.0
        bv[:, i] = np.exp(-(((grid - center) / width) ** 2))
    bv = bv / (bv.sum(axis=1, keepdims=True) + 1e-6)
    return bv


def _prep_consts(cp0, bw0, sw0, imp0, cp1, bw1, sw1, imp1):
    f8 = np.float64
    bv = _basis_table().astype(f8)
    d1, d2 = bv[1] - bv[0], bv[2] - bv[1]

    def fold(cp, bw, sw, imp):
        T = imp.astype(f8)[:, :, None] * sw.astype(f8)[:, :, None] * cp.astype(f8)
        Wb = imp.astype(f8) * bw.astype(f8)
        return Wb, T @ d1, T @ d2, (T @ bv[1]).sum(axis=0)

    Wb0, P10, P20, b1 = fold(cp0, bw0, sw0, imp0)
    Wb1, P11, P21, b2 = fold(cp1, bw1, sw1, imp1)
    bias2_eff = b2 + b1 @ P21

    w1 = np.stack([Wb0, P10, P20 - P10], axis=0)  # [3, 64, 128] lhsT chunks
    w1 = np.concatenate([w1, w1], axis=1)  # duplicate rows for partitions 64-127
    w1 = np.ascontiguousarray(w1.transpose(1, 0, 2)).reshape(128, 384)
    w2 = np.stack([Wb1, P11, P21 - P11], axis=0)  # [3, 128, 64] rhs chunks
    w2 = np.ascontiguousarray(w2.transpose(1, 0, 2)).reshape(128, 192)

    return {
        "wpk": np.concatenate([w1, w2], axis=1).astype(BF),  # [128, 576]
        "spk": np.stack(
            [b1, -1.0 - b1, 1.0 - b1, -b1], axis=1
        ).astype(np.float32),  # [128, 4] = b1|s1|s2|nb1
        "b2row": np.tile(bias2_eff, 8).astype(BF).reshape(1, 512),
    }


def _build(rows):
    assert rows % MACRO == 0
    nc = bacc.Bacc(
        "TRN2",
        target_bir_lowering=False,
        debug=False,
        enable_asserts=False,
        num_devices=N_CORES,
    )
    xd = nc.dram_tensor("x", [rows, D0], F32, kind="ExternalInput")
    wpkd = nc.dram_tensor("wpk", [128, 576], BF16, kind="ExternalInput")
    spkd = nc.dram_tensor("spk", [128, 4], F32, kind="ExternalInput")
    b2d = nc.dram_tensor("b2row", [1, 512], BF16, kind="ExternalInput")
    outd = nc.dram_tensor("out", [rows, D2], F32, kind="ExternalOutput")

    n_macro = rows // MACRO
    MAX, MIN = mybir.AluOpType.max, mybir.AluOpType.min
    SILU = mybir.ActivationFunctionType.Silu

    with tile.TileContext(nc) as tc, ExitStack() as ctx:
        consts = ctx.enter_context(tc.tile_pool(name="consts", bufs=1))
        xin = ctx.enter_context(tc.tile_pool(name="xin", bufs=8))
        f1 = ctx.enter_context(tc.tile_pool(name="f1", bufs=3))
        f2 = ctx.enter_context(tc.tile_pool(name="f2", bufs=2))
        osb = ctx.enter_context(tc.tile_pool(name="osb", bufs=3))
        ps_x = ctx.enter_context(tc.tile_pool(name="ps_x", bufs=1, space="PSUM"))
        ps_h = ctx.enter_context(tc.tile_pool(name="ps_h", bufs=2, space="PSUM"))
        ps_o = ctx.enter_context(tc.tile_pool(name="ps_o", bufs=3, space="PSUM"))

        wpk = consts.tile([128, 576], BF16)
        nc.sync.dma_start(wpk, wpkd.ap())
        spk = consts.tile([128, 4], F32)
        nc.sync.dma_start(spk, spkd.ap())
        b2r = consts.tile([1, 512], BF16)
        nc.sync.dma_start(b2r, b2d.ap())

        # PE pre-warm: dummy matmuls on a memset tile (no iota dependency)
        # while DMAs land, so the HAM clock gate opens (1.2 -> 2.4 GHz)
        # before the first real matmul issues.
        wz = consts.tile([128, 128], BF16)
        nc.vector.memset(wz, 0.0)
        warm = ps_o.tile([128, 8, 64], F32, tag="po")

        def warm_pe(n):
            for _ in range(n):
                nc.tensor.matmul(warm[:, 0:2], wz, wz, start=True, stop=True)

        warm_pe(18)

        ident = consts.tile([128, 128], BF16)
        make_identity(nc, ident)
        ones = consts.tile([1, 128], BF16)
        nc.vector.memset(ones, 1.0)
        b1, s1, s2, nb1 = (spk[:, i : i + 1] for i in range(4))
        w1c = [wpk[:, c * 128 : (c + 1) * 128] for c in range(3)]
        w2c = [wpk[:, 384 + c * 64 : 384 + (c + 1) * 64] for c in range(3)]

        xts, pxs, u1s, sl1s, c1s, hs = {}, {}, {}, {}, {}, {}

        def xt_dma_single(m):
            # Macros 0/1 load alone (2 KB descriptors) so the very first
            # transpose waits on a 256 KB transfer, not 512 KB.
            # xt[p, j, f] = x[base + 8p + j, f]
            xt = xin.tile([128, 8, 64], BF16, tag="xts")
            src = bass.AP(
                xd, m * MACRO * 64,
                [[8 * 64, 128], [64, 8], [1, 64]],
            )
            nc.gpsimd.dma_start(xt, src)
            xts[("s", m)] = xt

        def xt_dma_pair(k):
            # Load macros 2k and 2k+1 in one SWDGE: partition p reads 16
            # consecutive rows of x = one contiguous 4 KB descriptor (cast to
            # 2 KB bf16).  xt[p, j, f] = x[pair_base + 16p + j, f]; macro
            # 2k uses j=0..7, macro 2k+1 uses j=8..15.  Row "block" g of a
            # macro holds rows {base + 16p + g} (stride-16 permutation),
            # undone by the out DMA with equally-contiguous descriptors.
            xt = xin.tile([128, 16, 64], BF16, tag="xt")
            src = bass.AP(
                xd, k * 2 * MACRO * 64,
                [[16 * 64, 128], [64, 16], [1, 64]],
            )
            nc.gpsimd.dma_start(xt, src)
            xts[k] = xt

        def stage_transpose(m):
            # px[p,q,:] partitions 0-63 = feats of block 2q, partitions
            # 64-127 = feats of block 2q+1; free = 128 rows
            px = ps_x.tile([128, 4, 128], BF16, tag="px")
            if m < 2:
                xt, off = xts[("s", m)], 0
            else:
                xt, off = xts[m // 2], 8 * (m % 2)
            for q in range(4):
                nc.tensor.transpose(px[:, q], xt[:, off + 2 * q : off + 2 * q + 2], ident)
            if m < 2:
                del xts[("s", m)]
            elif m % 2 == 1:
                del xts[m // 2]
            pxs[m] = px

        def stage_feats1(m):
            px = pxs.pop(m)
            u1 = f1.tile([128, 4, 128], BF16, tag="u1")
            nc.vector.tensor_scalar(u1, px, -1.0, 1.0, op0=MAX, op1=MIN)
            sl1 = f1.tile([128, 4, 128], BF16, tag="sl1")
            nc.scalar.activation(sl1, px, SILU)
            c1 = f1.tile([128, 4, 128], BF16, tag="c1")
            nc.vector.tensor_scalar_max(c1, u1, 0.0)
            u1s[m], sl1s[m], c1s[m] = u1, sl1, c1

        def stage_l1(m):
            # L1: two concurrent 64-contraction row-group streams (A=even
            # blocks on partitions 0-63 of h[:, 0:512], B=odd on 512:1024)
            h = ps_h.tile([128, 1024], F32, tag="h")
            u1, sl1, c1 = u1s.pop(m), sl1s.pop(m), c1s.pop(m)
            for i, (c, ft) in enumerate([(1, u1), (0, sl1), (2, c1)]):
                nc.tensor.matmul(
                    h[:, 0:512], w1c[c][0:64], ft[0:64], start=(i == 0), stop=(i == 2)
                )
                nc.tensor.matmul(
                    h[:, 512:1024], w1c[c][64:128], ft[64:128],
                    start=(i == 0), stop=(i == 2),
                )
            hs[m] = h

        maps2 = {}

        def stage_maps2(m):
            h = hs.pop(m)
            # L2 feature maps; sl2 is one 1024-wide op spanning both h banks
            sl2 = f2.tile([128, 1024], BF16, tag="sl2")
            nc.scalar.activation(sl2, h, SILU, bias=b1)
            u2 = f2.tile([128, 1024], BF16, tag="u2")
            nc.vector.tensor_scalar(u2[:, 0:512], h[:, 0:512], s1, s2, op0=MAX, op1=MIN)
            nc.vector.tensor_scalar(u2[:, 512:1024], h[:, 512:1024], s1, s2, op0=MAX, op1=MIN)
            c2 = f2.tile([128, 1024], BF16, tag="c2")
            nc.vector.tensor_scalar_max(c2, u2, nb1)
            maps2[m] = (sl2, u2, c2)

        pos = {}

        def stage_l2mm(m):
            sl2, u2, c2 = maps2.pop(m)
            # bias init via K=1 ones-matmul (sets has_written on the whole
            # bank so the 24 block matmuls accumulate with start=False)
            po = ps_o.tile([128, 8, 64], F32, tag="po")
            nc.tensor.matmul(po, ones, b2r, start=True, stop=False)
            # chunk order by map readiness: u2 first, then sl2, then c2
            for ci, ft in [(1, u2), (0, sl2), (2, c2)]:
                for g in range(8):
                    off = (g % 2) * 512 + (g // 2) * 128
                    nc.tensor.matmul(
                        po[:, g],
                        ft[:, off : off + 128],
                        w2c[ci],
                        start=False,
                        stop=(ci == 2 and g == 7),
                    )
            pos[m] = po

        def stage_out(m):
            po = pos.pop(m)
            ot = osb.tile([128, 8, 64], F32, tag="ot")
            nc.scalar.copy(ot, po)
            # per-partition 8 consecutive out rows = one contiguous 2 KB
            # descriptor; row base is 8p (single-loaded macros) or 16p (pairs)
            if m < 2:
                dst = bass.AP(outd, m * MACRO * 64, [[8 * 64, 128], [64, 8], [1, 64]])
            else:
                dst = bass.AP(
                    outd, ((m // 2) * 2 * MACRO + 8 * (m % 2)) * 64,
                    [[16 * 64, 128], [64, 8], [1, 64]],
                )
            nc.sync.dma_start(dst, ot)

        # Software-pipelined main loop.  Macro 0 runs unskewed (its stage-B
        # right after its L1) so the first output leaves early; the pipeline
        # bubble that builds the steady skew-1 lands in iteration 1, where it
        # is cheap.  Warm matmuls interleave with the first iterations to
        # keep the PE clock hot through fill (PE idle gaps reset the HAM
        # clock ramp back to 1.2 GHz).
        assert n_macro % 2 == 0
        # Macros 0 and 1 load alone; the rest in pairs, prefetched a few
        # iterations ahead so the in-queue drains early.
        xt_dma_single(0)
        xt_dma_single(1)
        for k in range(1, 4):
            xt_dma_pair(k)
        stage_transpose(0)
        warm_pe(8)
        stage_feats1(0)
        warm_pe(6)
        warm_fill = {1: (4, 10), 2: (3, 4), 3: (2, 3)}
        for m in range(n_macro):
            if m in (2, 4, 6, 8):
                xt_dma_pair(m // 2 + 3)
            if m + 1 < n_macro:
                stage_transpose(m + 1)
            if m in warm_fill:
                warm_pe(warm_fill[m][0])
            if m == 0:
                # macro 0 unskewed, and its maps2 emitted before feats1(1)
                # so the DVE/ACT queues reach it without blocking on T(1)
                warm_pe(4)
                stage_l1(0)
                stage_maps2(0)
                warm_pe(6)
                stage_feats1(1)
                stage_l2mm(0)
                continue
            if m + 1 < n_macro:
                stage_feats1(m + 1)
            stage_l1(m)
            if m in warm_fill:
                warm_pe(warm_fill[m][1])
            if m >= 2:
                stage_maps2(m - 1)
                stage_l2mm(m - 1)
                stage_out(m - 2)
        stage_maps2(n_macro - 1)
        stage_l2mm(n_macro - 1)
        stage_out(n_macro - 2)
        stage_out(n_macro - 1)

    nc.compile()
    return nc


def _get_nc(rows):
    if rows not in _nc_cache:
        _nc_cache[rows] = _build(rows)
    return _nc_cache[rows]


def kernel(x, cp0, bw0, sw0, imp0, cp1, bw1, sw1, imp1, _trace=False, _trace_kwargs=None):
    x = np.ascontiguousarray(np.asarray(x, dtype=np.float32))
    consts = _prep_consts(
        *[np.asarray(a, dtype=np.float32) for a in (cp0, bw0, sw0, imp0, cp1, bw1, sw1, imp1)]
    )
    rows = x.shape[0] // N_CORES
    nc = _get_nc(rows)
    in_maps = []
    for i in range(N_CORES):
        m = dict(consts)
        m["x"] = x[i * rows : (i + 1) * rows]
        in_maps.append(m)
    res = run_bass_kernel_spmd(
        nc, in_maps, list(range(N_CORES)), trace=_trace, **(_trace_kwargs or {})
    )
    out = np.concatenate([res.results[i]["out"] for i in range(N_CORES)], axis=0)
    if _trace:
        return out, res
    return out
